# revision 19
# baseline (speedup 1.0000x reference)
"""CapsuleLayer (dynamic routing) Trainium2 kernel.

Problem: x[256,1152,8], W[1152,10,8,16] ->
  u_hat = einsum('bik,ijkd->bijd', x, W); 3 routing iterations -> out [256,10,16]

Strategy (8 cores, data-parallel over batch, W replicated on every core):
  Per core: 32 samples. u_hat (5.9M elems) lives entirely in SBUF as bf16 with
  layout [partition = (i%4)*32 + b, free = (I=i//4 in 0..288, d, j)].

  The u_hat-producing matmuls, their PSUM->SBUF evacuation, and routing
  iteration 1 are fused into ONE software-pipelined loop over 12 chunks:
    body b: PE burst  = 24 u-matmuls(chunk b) + 24 s-fold matmuls(chunk b-3)
            evac(b)   = 8 PSUM->SBUF bf16 copies on ACT (+Pool tail)
            front(b-1)= A-mul u*vrep + d-tree + r write + exp
            back(b-2) = Z, 1/Z, c=e*Zr, B-mul c*u -> t2
  This keeps the PE stream dense (p-state ramp), lets ACT absorb the whole
  evacuation (it is otherwise idle during routing), and leaves DVE/Pool for
  the irreducible tensor-tensor work. s0 = 0.1*sum_i u_hat comes from a
  K=9216 accumulated matmul interleaved into the first two PE bursts, so
  v0/vrep0 are ready before front(0).

  Iteration 2 is the classic 2-stage pipeline (front/back, LAG=1) with a
  tuned DVE/Pool split (h2 level + c on Pool, B-mul head blocks on Pool).
  Squash uses Newton rsqrt via bitcast seed; v broadcast to 128 partitions
  via a tiny PE matmul.
"""

import sys

if "/opt/trn_rl_repo" not in sys.path:
    sys.path.insert(0, "/opt/trn_rl_repo")

import numpy as np
import ml_dtypes

BF16 = ml_dtypes.bfloat16

B, IC, ID, OC, OD = 256, 1152, 8, 10, 16
NCORES = 8
BC = B // NCORES  # 32 samples per core
NB = IC // 4  # 288 blocks of 4 input capsules
NW, WB = 4, NB // 4  # 4 windows x 72 blocks
NCHUNK, CB = 12, NB // 12  # routing chunks
NG = CB // 3  # 8 psum groups of 3 I-blocks per chunk
JD = OC * OD  # 160, stored (d, j): offset = d*OC + j
KQ = IC * ID // 128  # 72 chunks of the 9216 contraction
EPS = 1e-7

# tuning knobs
EVAC_PAT = "AAAAAAAA"  # engine per psum group: A=ACT, D=DVE (GPSIMD can't read PSUM)
PI1 = 8  # B-mul head blocks on Pool, fused loop (iter 1)
PI2 = 8  # B-mul head blocks on Pool, iter 2

_CACHE = {}


def _build_nc():
    from contextlib import ExitStack

    import concourse.bass as bass
    import concourse.tile as tile
    from concourse import bacc, mybir

    dt = mybir.dt
    f32 = dt.float32
    bf = dt.bfloat16
    i32 = dt.int32
    X = mybir.AxisListType.X
    AF = mybir.ActivationFunctionType
    OP = mybir.AluOpType

    nc = bacc.Bacc("TRN2", target_bir_lowering=False, debug=False, num_devices=NCORES)

    d_xblk = nc.dram_tensor("xblk", [128, WB, 128], bf, kind="ExternalInput")
    d_wrhs = nc.dram_tensor("wrhs", [128, WB, JD], bf, kind="ExternalInput")
    d_w9 = nc.dram_tensor("w9", [128, KQ, JD], bf, kind="ExternalInput")
    d_xt9 = nc.dram_tensor("xt9", [128, KQ, BC], bf, kind="ExternalInput")
    d_ones = nc.dram_tensor("onesb", [128, BC], bf, kind="ExternalInput")
    d_bcw = nc.dram_tensor("bcw", [BC, 128], bf, kind="ExternalInput")
    d_id80 = nc.dram_tensor("id80", [80, 80], f32, kind="ExternalInput")
    d_y = nc.dram_tensor("y", [BC, OC, OD], f32, kind="ExternalOutput")

    def ap_of(t, free_pairs, extra_off=0):
        """View tile t with custom free-dim [step, count] pairs (partition dim kept)."""
        base = t[:]
        return bass.AP(
            tensor=base.tensor,
            offset=base.offset + extra_off,
            ap=[base.ap[0]] + free_pairs,
        )

    with ExitStack() as ctx:
        tc = ctx.enter_context(tile.TileContext(nc))
        statics = ctx.enter_context(tc.tile_pool(name="statics", bufs=1))
        scratch = ctx.enter_context(tc.tile_pool(name="scratch", bufs=4))
        bigs = ctx.enter_context(tc.tile_pool(name="bigs", bufs=1))
        smalls = ctx.enter_context(tc.tile_pool(name="smalls", bufs=2))
        sm2 = ctx.enter_context(tc.tile_pool(name="sm2", bufs=3))
        pe_pool = ctx.enter_context(tc.tile_pool(name="pe", bufs=4, space="PSUM"))
        ps_pool = ctx.enter_context(tc.tile_pool(name="ps", bufs=1, space="PSUM"))
        pv_pool = ctx.enter_context(tc.tile_pool(name="pv", bufs=1, space="PSUM"))

        # ---------- statics in (s0 inputs first so the prelude can start) ----
        xblk = statics.tile([128, WB, 128], bf)
        wrhs = statics.tile([128, WB, JD], bf)
        onesb = statics.tile([128, BC], bf)
        bcw = statics.tile([BC, 128], bf)
        w9 = statics.tile([128, KQ, JD], bf)
        xt9 = statics.tile([128, KQ, BC], bf)
        nc.sync.dma_start(xt9[:], d_xt9[:])
        third = WB // 3
        for c0 in range(0, WB, third):
            cs = slice(c0, c0 + third)
            nc.sync.dma_start(w9[:, cs], d_w9[:, cs])
        for w in range(NW):
            sl = slice(32 * w, 32 * w + 32)
            if w == 0:
                for c0 in range(0, WB, third):
                    cs = slice(c0, c0 + third)
                    nc.sync.dma_start(xblk[sl, cs], d_xblk[sl, cs])
                    nc.sync.dma_start(wrhs[sl, cs], d_wrhs[sl, cs])
            else:
                nc.sync.dma_start(xblk[sl], d_xblk[sl])
                nc.sync.dma_start(wrhs[sl], d_wrhs[sl])
        nc.sync.dma_start(onesb[:], d_ones[:])
        nc.sync.dma_start(bcw[:], d_bcw[:])
        id80 = statics.tile([80, 80], f32)
        nc.sync.dma_start(id80[:], d_id80[:])

        # ---------- persistent big tensors ----------
        uhat = bigs.tile([128, NB, JD], bf)  # free (I, d, j)
        r_bf = bigs.tile([128, NB, OC], bf)  # logits
        vrep = bigs.tile([128, JD], bf, tag="vrep")

        # ---------- squash helper ----------
        nw_magic = smalls.tile([BC, OC], i32, tag="sq_magic")
        nc.vector.memset(nw_magic[:], 0x5F3759DF)
        nw_onei = smalls.tile([BC, OC], i32, tag="sq_onei")
        nc.vector.memset(nw_onei[:], 1)

        def squash(s_psum, pre_scale):
            """v = squash(pre_scale * s_psum); returns (v_f32, v_bf16) [BC, JD]."""
            s = smalls.tile([BC, JD], f32, tag="sq_s")
            nc.vector.tensor_scalar_mul(s[:], s_psum[:], pre_scale)
            sq = smalls.tile([BC, JD], f32, tag="sq_sq")
            nc.vector.tensor_mul(sq[:], s[:], s[:])
            S = smalls.tile([BC, OC], f32, tag="sq_S")
            sq_v = ap_of(sq, [[1, OC], [OC, OD]])
            nc.vector.reduce_sum(S[:], sq_v, axis=X)
            Sp = smalls.tile([BC, OC], f32, tag="sq_Sp")
            nc.vector.tensor_scalar_add(Sp[:], S[:], EPS)
            half = smalls.tile([BC, OC], i32, tag="sq_half")
            nc.vector.tensor_tensor(
                half[:], Sp[:].bitcast(i32), nw_onei[:], op=OP.arith_shift_right
            )
            y = smalls.tile([BC, OC], f32, tag="sq_y")
            nc.vector.tensor_tensor(
                y[:].bitcast(i32), nw_magic[:], half[:], op=OP.subtract
            )
            for it in range(2):
                t0 = smalls.tile([BC, OC], f32, tag="sq_t0")
                nc.vector.tensor_mul(t0[:], y[:], y[:])
                t1 = smalls.tile([BC, OC], f32, tag="sq_t1")
                nc.vector.tensor_mul(t1[:], t0[:], Sp[:])
                t2 = smalls.tile([BC, OC], f32, tag="sq_t2")
                nc.vector.tensor_scalar(t2[:], t1[:], -0.5, 1.5, op0=OP.mult, op1=OP.add)
                y2 = smalls.tile([BC, OC], f32, tag="sq_y")
                nc.vector.tensor_mul(y2[:], y[:], t2[:])
                y = y2
            g = smalls.tile([BC, OC], f32, tag="sq_g")
            nc.vector.tensor_scalar_add(g[:], S[:], 1.0)
            gr = smalls.tile([BC, OC], f32, tag="sq_gr")
            nc.vector.reciprocal(gr[:], g[:])
            sc = smalls.tile([BC, OC], f32, tag="sq_sc")
            nc.vector.tensor_mul(sc[:], S[:], gr[:])
            sc2 = smalls.tile([BC, OC], f32, tag="sq_sc2")
            nc.vector.tensor_mul(sc2[:], sc[:], y[:])
            v = smalls.tile([BC, JD], f32, tag="sq_v")
            v_3d = ap_of(v, [[1, OC], [OC, OD]])
            s_3d = ap_of(s, [[1, OC], [OC, OD]])
            sc_3d = ap_of(sc2, [[1, OC], [0, OD]])
            nc.vector.tensor_mul(v_3d, s_3d, sc_3d)
            v_b = smalls.tile([BC, JD], bf, tag="sq_vb")
            nc.vector.tensor_copy(v_b[:], v[:])
            return v, v_b

        def make_vrep(v_b):
            vp = pv_pool.tile([128, JD], f32, tag="vp")
            nc.tensor.matmul(vp[:], bcw[:], v_b[:], start=True, stop=True)
            nc.vector.tensor_copy(vrep[:], vp[:])

        def fold_sp(sacch):
            """((dj-half, b) PSUM accumulator pair) -> [32, 160] PSUM via
            two PE transposes (copy through SBUF f32). The halves live in
            separate PSUM tiles: interleaved accumulation chains in one bank
            corrupt each other."""
            ssb = smalls.tile([80, 2, 32], f32, tag="ssb")
            nc.vector.tensor_copy(ssb[:, 0, :], sacch[0][:])
            nc.vector.tensor_copy(ssb[:, 1, :], sacch[1][:])
            sp = pv_pool.tile([BC, JD], f32, tag="sp")
            for h in (0, 1):
                nc.tensor.transpose(
                    sp[:, 80 * h : 80 * h + 80], ssb[:, h, :], id80[:]
                )
            return sp

        def sacc_pair():
            return (
                ps_pool.tile([80, BC], f32, tag="sacc_a", name="sacc_a"),
                ps_pool.tile([80, BC], f32, tag="sacc_b", name="sacc_b"),
            )

        # ---------- fused phase 1 + iteration 1 ----------
        # s-accumulators live as [80, 2, 32] = ((d,j) half, half, b): the fold
        # matmuls use t2/w9 as lhsT and a 32-wide rhs, so each feeds 32 rows
        # instead of 160 (2.5x less PE time); one transpose pair per squash
        # converts back to [32, 160].
        s0h = sacc_pair()
        sacc1 = None
        st_e = {}
        st_t1 = {}
        st_t2 = {}

        def umm_group(q, g, s0_span):
            """3 u-matmuls into one PSUM group + optional s0 matmuls; evac."""
            pt = pe_pool.tile([128, 3, JD], f32)
            for s in range(3):
                I = q * CB + 3 * g + s
                w, step = divmod(I, WB)
                sl = slice(32 * w, 32 * w + 32)
                nc.tensor.matmul(
                    pt[:, s, :],
                    xblk[sl, step, :],
                    wrhs[sl, step, :],
                    start=True,
                    stop=True,
                    tile_position=(32 * w, 0),
                )
            for kq in s0_span:
                for h in (0, 1):
                    nc.tensor.matmul(
                        s0h[h][:],
                        w9[:, kq, 80 * h : 80 * h + 80],
                        xt9[:, kq, :],
                        start=(kq == 0),
                        stop=(kq == KQ - 1),
                    )
            dst = uhat[:, q * CB + 3 * g : q * CB + 3 * g + 3, :]
            eng = EVAC_PAT[g]
            if eng == "A":
                nc.scalar.copy(dst, pt[:])
            else:
                nc.vector.tensor_copy(dst, pt[:])

        def front_A(q):
            I0 = q * CB
            t1 = scratch.tile([128, CB, JD], bf, tag="sc")
            vr_b = ap_of(vrep, [[0, CB], [1, JD]])
            nc.vector.tensor_mul(t1[:], uhat[:, I0 : I0 + CB, :], vr_b)
            st_t1[q] = t1

        def front_tree(q):
            t1v = st_t1[q][:].rearrange("p c (d j) -> p c d j", d=OD)
            nc.vector.tensor_add(t1v[:, :, 0:8, :], t1v[:, :, 0:8, :], t1v[:, :, 8:16, :])
            nc.vector.tensor_add(t1v[:, :, 0:4, :], t1v[:, :, 0:4, :], t1v[:, :, 4:8, :])

        def front_tail(q, t):
            t1v = st_t1.pop(q)[:].rearrange("p c (d j) -> p c d j", d=OD)
            nc.gpsimd.tensor_add(t1v[:, :, 0:2, :], t1v[:, :, 0:2, :], t1v[:, :, 2:4, :])
            nc.gpsimd.tensor_add(t1v[:, :, 0:1, :], t1v[:, :, 0:1, :], t1v[:, :, 1:2, :])
            rsl = r_bf[:, q * CB : q * CB + CB, :]
            if t == 1:
                nc.gpsimd.tensor_copy(rsl, t1v[:, :, 0, :])
            else:
                nc.gpsimd.tensor_add(rsl, rsl, t1v[:, :, 0, :])
            e_t = sm2.tile([128, CB, OC], bf, tag="e")
            nc.scalar.activation(e_t[:], rsl, AF.Exp)
            st_e[q] = e_t

        def back_zrec(q):
            e_t = st_e[q]
            z_t = sm2.tile([128, CB], f32, tag="z")
            nc.vector.reduce_sum(z_t[:], e_t[:], axis=X)
            nc.vector.reciprocal(z_t[:], z_t[:])
            return z_t

        def back_c(q, z_t):
            e_t = st_e.pop(q)
            c_t = sm2.tile([128, CB, OC], bf, tag="c")
            z_b = ap_of(z_t, [[1, CB], [0, OC]])
            nc.gpsimd.tensor_mul(c_t[:], e_t[:], z_b)
            return c_t

        def back_Bhead(q, c_t, pi):
            t2 = scratch.tile([128, CB, JD], bf, tag="sc")
            u4 = uhat[:, q * CB : q * CB + CB, :].rearrange("p c (d j) -> p c d j", d=OD)
            t24 = t2[:].rearrange("p c (d j) -> p c d j", d=OD)
            c_p = ap_of(c_t, [[OC, pi], [0, OD], [1, OC]])
            nc.gpsimd.tensor_mul(t24[:, 0:pi], u4[:, 0:pi], c_p)
            st_t2[q] = (t2, c_t)

        def back_Btail(q, pi):
            t2, c_t = st_t2[q]
            u4 = uhat[:, q * CB : q * CB + CB, :].rearrange("p c (d j) -> p c d j", d=OD)
            t24 = t2[:].rearrange("p c (d j) -> p c d j", d=OD)
            c_d = ap_of(c_t, [[OC, CB - pi], [0, OD], [1, OC]], extra_off=pi * OC)
            nc.vector.tensor_mul(t24[:, pi:CB], u4[:, pi:CB], c_d)

        def s_folds(q, sacc):
            t2, _ = st_t2.pop(q)
            for s in range(CB):
                gi = q * CB + s
                for h in (0, 1):
                    nc.tensor.matmul(
                        sacc[h][:],
                        t2[:, s, 80 * h : 80 * h + 80],
                        onesb[:],
                        start=(gi == 0),
                        stop=(gi == NB - 1),
                    )

        # Prelude BEFORE any u-matmul is emitted: the Tile scheduler pops the
        # lowest-priority READY instruction at every engine-idle, so the
        # s0 -> squash -> vrep chain must outrank the evac-gated u-matmul
        # backlog or the fronts stall behind ~50us of PE flood.
        for kq in range(KQ):
            for h in (0, 1):
                nc.tensor.matmul(
                    s0h[h][:],
                    w9[:, kq, 80 * h : 80 * h + 80],
                    xt9[:, kq, :],
                    start=(kq == 0),
                    stop=(kq == KQ - 1),
                )
        v_f, v_b = squash(fold_sp(s0h), 0.1)
        make_vrep(v_b)
        sacc1 = sacc_pair()

        # body b: PE u-mms(b)+evac(b) [ACT]; DVE [A(b-2), Z(b-3), rec(b-3),
        # B-tail(b-3), h8(b-2), h4(b-2)]; Pool [c(b-3), B-head(b-3), h2(b-2),
        # h1(b-2), r(b-2)]; ACT exp(b-2); PE s-folds(b-3). Every cross-engine
        # input is >= 1 body old except rec->c and c->B-tail (short waits).
        for body in range(NCHUNK + 3):
            if body < NCHUNK:
                for g in range(NG):
                    umm_group(body, g, ())
            if 2 <= body <= NCHUNK + 1:
                front_A(body - 2)
            if 3 <= body <= NCHUNK + 2:
                qb = body - 3
                z_t = back_zrec(qb)
                c_t = back_c(qb, z_t)
                back_Bhead(qb, c_t, PI1)
                back_Btail(qb, PI1)
                s_folds(qb, sacc1)
            if 2 <= body <= NCHUNK + 1:
                front_tree(body - 2)
                front_tail(body - 2, t=1)

        v_f, v_b = squash(fold_sp(sacc1), 1.0)
        make_vrep(v_b)

        # ---------- iteration 2 ----------
        sacc2 = sacc_pair()
        for body in range(NCHUNK + 2):
            if body < NCHUNK:
                front_A(body)
            if body >= 2:
                qb = body - 2
                z_t = back_zrec(qb)
                c_t = back_c(qb, z_t)
                back_Bhead(qb, c_t, PI2)
                back_Btail(qb, PI2)
                s_folds(qb, sacc2)
            if body < NCHUNK:
                front_tree(body)
                front_tail(body, t=2)

        v_f, v_b = squash(fold_sp(sacc2), 1.0)

        # ---------- output ----------
        stage = smalls.tile([BC, JD], f32, tag="stage")
        st_v = ap_of(stage, [[OD, OC], [1, OD]])  # (j outer, d inner) dense
        vf_v = ap_of(v_f, [[1, OC], [OC, OD]])
        nc.vector.tensor_copy(st_v, vf_v)
        nc.sync.dma_start(d_y[:], stage[:].rearrange("p (j d) -> p j d", j=OC))

    nc.compile()
    return nc


def _prep_host(x, W):
    """Build per-core input maps. x [256,1152,8] f32, W [1152,10,8,16] f32."""
    Wv = W.reshape(NW, WB, 4, OC, ID, OD)  # [w, step, ip, j, k, d]
    wrhs = np.ascontiguousarray(
        Wv.transpose(0, 2, 4, 1, 5, 3).reshape(128, WB, JD)
    ).astype(BF16)
    Wf = W.transpose(0, 2, 3, 1).reshape(IC * ID, OD, OC)  # [(i,k), d, j]
    w9 = np.ascontiguousarray(
        Wf.reshape(KQ, 128, OD, OC).transpose(1, 0, 2, 3).reshape(128, KQ, JD)
    ).astype(BF16)
    onesb = np.zeros((128, BC), dtype=BF16)
    onesb[np.arange(128), np.arange(128) % 32] = 1
    bcw = np.zeros((BC, 128), dtype=BF16)
    bcw[np.arange(128) % 32, np.arange(128)] = 1
    id80 = np.eye(80, dtype=np.float32)

    in_maps = []
    for c in range(NCORES):
        xc = x[c * BC : (c + 1) * BC]  # [32, 1152, 8]
        xv = xc.reshape(BC, NW, WB, 4, ID)  # [b, w, step, ip, k]
        xa = np.zeros((NW, 4, ID, WB, 4, BC), dtype=BF16)  # [w, ip, k, step, ipp, b]
        for ip in range(4):
            xa[:, ip, :, :, ip, :] = xv[:, :, :, ip, :].transpose(1, 3, 2, 0)
        xblk = xa.reshape(128, WB, 128)
        xf = xc.transpose(1, 2, 0).reshape(IC * ID, BC)  # [(i,k), b]
        xt9 = np.ascontiguousarray(
            xf.reshape(KQ, 128, BC).transpose(1, 0, 2)
        ).astype(BF16)
        in_maps.append(
            {
                "xblk": xblk,
                "wrhs": wrhs,
                "w9": w9,
                "xt9": xt9,
                "onesb": onesb,
                "bcw": bcw,
                "id80": id80,
            }
        )
    return in_maps


def kernel(x, W, _trace=False, _trace_kwargs=None):
    from concourse.bass_utils import run_bass_kernel_spmd

    x = np.asarray(x, dtype=np.float32)
    W = np.asarray(W, dtype=np.float32)
    if "nc" not in _CACHE:
        _CACHE["nc"] = _build_nc()
    nc = _CACHE["nc"]
    in_maps = _prep_host(x, W)
    res = run_bass_kernel_spmd(
        nc,
        in_maps,
        core_ids=list(range(NCORES)),
        trace=_trace,
        **(_trace_kwargs or {}),
    )
    _CACHE["last_results"] = res
    out = np.concatenate([res.results[c]["y"] for c in range(NCORES)], axis=0)
    return out


# revision 20
# speedup vs baseline: 1.1568x; 1.1568x over previous
"""CapsuleLayer (dynamic routing) Trainium2 kernel.

Problem: x[256,1152,8], W[1152,10,8,16] ->
  u_hat = einsum('bik,ijkd->bijd', x, W); 3 routing iterations -> out [256,10,16]

Strategy (8 cores, data-parallel over batch, W replicated on every core):
  Per core: 32 samples. u_hat (5.9M elems) lives entirely in SBUF as bf16 with
  layout [partition = (i%4)*32 + b, free = (I=i//4 in 0..288, d, j)].

  The u_hat-producing matmuls, their PSUM->SBUF evacuation, and routing
  iteration 1 are fused into ONE software-pipelined loop over 12 chunks:
    body b: PE burst  = 24 u-matmuls(chunk b) + 24 s-fold matmuls(chunk b-3)
            evac(b)   = 8 PSUM->SBUF bf16 copies on ACT (+Pool tail)
            front(b-1)= A-mul u*vrep + d-tree + r write + exp
            back(b-2) = Z, 1/Z, c=e*Zr, B-mul c*u -> t2
  This keeps the PE stream dense (p-state ramp), lets ACT absorb the whole
  evacuation (it is otherwise idle during routing), and leaves DVE/Pool for
  the irreducible tensor-tensor work. s0 = 0.1*sum_i u_hat comes from a
  K=9216 accumulated matmul interleaved into the first two PE bursts, so
  v0/vrep0 are ready before front(0).

  Iteration 2 is the classic 2-stage pipeline (front/back, LAG=1) with a
  tuned DVE/Pool split (h2 level + c on Pool, B-mul head blocks on Pool).
  Squash uses Newton rsqrt via bitcast seed; v broadcast to 128 partitions
  via a tiny PE matmul.
"""

import sys

if "/opt/trn_rl_repo" not in sys.path:
    sys.path.insert(0, "/opt/trn_rl_repo")

import numpy as np
import ml_dtypes

BF16 = ml_dtypes.bfloat16

B, IC, ID, OC, OD = 256, 1152, 8, 10, 16
NCORES = 8
BC = B // NCORES  # 32 samples per core
NB = IC // 4  # 288 blocks of 4 input capsules
NW, WB = 4, NB // 4  # 4 windows x 72 blocks
NCHUNK, CB = 12, NB // 12  # routing chunks
NG = CB // 3  # 8 psum groups of 3 I-blocks per chunk
JD = OC * OD  # 160, stored (d, j): offset = d*OC + j
KQ = IC * ID // 128  # 72 chunks of the 9216 contraction
EPS = 1e-7

# tuning knobs
EVAC_PAT = "AAAAAAAA"  # engine per psum group: A=ACT, D=DVE (GPSIMD can't read PSUM)
PI1 = 8  # B-mul head blocks on Pool, fused loop (iter 1)
PI2 = 8  # B-mul head blocks on Pool, iter 2

_CACHE = {}


def _build_nc():
    from contextlib import ExitStack

    import concourse.bass as bass
    import concourse.tile as tile
    from concourse import bacc, mybir

    dt = mybir.dt
    f32 = dt.float32
    bf = dt.bfloat16
    i32 = dt.int32
    X = mybir.AxisListType.X
    AF = mybir.ActivationFunctionType
    OP = mybir.AluOpType

    nc = bacc.Bacc("TRN2", target_bir_lowering=False, debug=False, num_devices=NCORES)

    d_xblk = nc.dram_tensor("xblk", [128, WB, 128], bf, kind="ExternalInput")
    d_wrhs = nc.dram_tensor("wrhs", [128, WB, JD], bf, kind="ExternalInput")
    d_w9 = nc.dram_tensor("w9", [128, KQ, JD], bf, kind="ExternalInput")
    d_xt9 = nc.dram_tensor("xt9", [128, KQ, BC], bf, kind="ExternalInput")
    d_ones = nc.dram_tensor("onesb", [128, BC], bf, kind="ExternalInput")
    d_bcw = nc.dram_tensor("bcw", [BC, 128], bf, kind="ExternalInput")
    d_id80 = nc.dram_tensor("id80", [80, 80], f32, kind="ExternalInput")
    d_y = nc.dram_tensor("y", [BC, OC, OD], f32, kind="ExternalOutput")

    def ap_of(t, free_pairs, extra_off=0):
        """View tile t with custom free-dim [step, count] pairs (partition dim kept)."""
        base = t[:]
        return bass.AP(
            tensor=base.tensor,
            offset=base.offset + extra_off,
            ap=[base.ap[0]] + free_pairs,
        )

    with ExitStack() as ctx:
        tc = ctx.enter_context(tile.TileContext(nc))
        statics = ctx.enter_context(tc.tile_pool(name="statics", bufs=1))
        scratch = ctx.enter_context(tc.tile_pool(name="scratch", bufs=4))
        bigs = ctx.enter_context(tc.tile_pool(name="bigs", bufs=1))
        smalls = ctx.enter_context(tc.tile_pool(name="smalls", bufs=2))
        sm2 = ctx.enter_context(tc.tile_pool(name="sm2", bufs=3))
        pe_pool = ctx.enter_context(tc.tile_pool(name="pe", bufs=4, space="PSUM"))
        ps_pool = ctx.enter_context(tc.tile_pool(name="ps", bufs=1, space="PSUM"))
        pv_pool = ctx.enter_context(tc.tile_pool(name="pv", bufs=1, space="PSUM"))

        # ---------- statics in (s0 inputs first so the prelude can start) ----
        xblk = statics.tile([128, WB, 128], bf)
        wrhs = statics.tile([128, WB, JD], bf)
        onesb = statics.tile([128, BC], bf)
        bcw = statics.tile([BC, 128], bf)
        w9 = statics.tile([128, KQ, JD], bf)
        xt9 = statics.tile([128, KQ, BC], bf)
        id80 = statics.tile([80, 80], f32)
        # tiny statics first: id80/bcw gate the s0 -> squash -> vrep chain,
        # and the serial DMA queue is ~25us long
        nc.sync.dma_start(id80[:], d_id80[:])
        nc.sync.dma_start(bcw[:], d_bcw[:])
        nc.sync.dma_start(onesb[:], d_ones[:])
        nc.sync.dma_start(xt9[:], d_xt9[:])
        third = WB // 3
        for c0 in range(0, WB, third):
            cs = slice(c0, c0 + third)
            nc.sync.dma_start(w9[:, cs], d_w9[:, cs])
        for w in range(NW):
            sl = slice(32 * w, 32 * w + 32)
            if w == 0:
                for c0 in range(0, WB, third):
                    cs = slice(c0, c0 + third)
                    nc.sync.dma_start(xblk[sl, cs], d_xblk[sl, cs])
                    nc.sync.dma_start(wrhs[sl, cs], d_wrhs[sl, cs])
            else:
                nc.sync.dma_start(xblk[sl], d_xblk[sl])
                nc.sync.dma_start(wrhs[sl], d_wrhs[sl])

        # ---------- persistent big tensors ----------
        uhat = bigs.tile([128, NB, JD], bf)  # free (I, d, j)
        r_bf = bigs.tile([128, NB, OC], bf)  # logits
        vrep = bigs.tile([128, JD], bf, tag="vrep")

        # ---------- squash helper ----------
        nw_magic = smalls.tile([BC, OC], i32, tag="sq_magic")
        nc.vector.memset(nw_magic[:], 0x5F3759DF)
        nw_onei = smalls.tile([BC, OC], i32, tag="sq_onei")
        nc.vector.memset(nw_onei[:], 1)

        def squash(s_psum, pre_scale):
            """v = squash(pre_scale * s_psum); returns (v_f32, v_bf16) [BC, JD]."""
            s = smalls.tile([BC, JD], f32, tag="sq_s")
            nc.vector.tensor_scalar_mul(s[:], s_psum[:], pre_scale)
            sq = smalls.tile([BC, JD], f32, tag="sq_sq")
            nc.vector.tensor_mul(sq[:], s[:], s[:])
            S = smalls.tile([BC, OC], f32, tag="sq_S")
            sq_v = ap_of(sq, [[1, OC], [OC, OD]])
            nc.vector.reduce_sum(S[:], sq_v, axis=X)
            Sp = smalls.tile([BC, OC], f32, tag="sq_Sp")
            nc.vector.tensor_scalar_add(Sp[:], S[:], EPS)
            half = smalls.tile([BC, OC], i32, tag="sq_half")
            nc.vector.tensor_tensor(
                half[:], Sp[:].bitcast(i32), nw_onei[:], op=OP.arith_shift_right
            )
            y = smalls.tile([BC, OC], f32, tag="sq_y")
            nc.vector.tensor_tensor(
                y[:].bitcast(i32), nw_magic[:], half[:], op=OP.subtract
            )
            for it in range(2):
                t0 = smalls.tile([BC, OC], f32, tag="sq_t0")
                nc.vector.tensor_mul(t0[:], y[:], y[:])
                t1 = smalls.tile([BC, OC], f32, tag="sq_t1")
                nc.vector.tensor_mul(t1[:], t0[:], Sp[:])
                t2 = smalls.tile([BC, OC], f32, tag="sq_t2")
                nc.vector.tensor_scalar(t2[:], t1[:], -0.5, 1.5, op0=OP.mult, op1=OP.add)
                y2 = smalls.tile([BC, OC], f32, tag="sq_y")
                nc.vector.tensor_mul(y2[:], y[:], t2[:])
                y = y2
            g = smalls.tile([BC, OC], f32, tag="sq_g")
            nc.vector.tensor_scalar_add(g[:], S[:], 1.0)
            gr = smalls.tile([BC, OC], f32, tag="sq_gr")
            nc.vector.reciprocal(gr[:], g[:])
            sc = smalls.tile([BC, OC], f32, tag="sq_sc")
            nc.vector.tensor_mul(sc[:], S[:], gr[:])
            sc2 = smalls.tile([BC, OC], f32, tag="sq_sc2")
            nc.vector.tensor_mul(sc2[:], sc[:], y[:])
            v = smalls.tile([BC, JD], f32, tag="sq_v")
            v_3d = ap_of(v, [[1, OC], [OC, OD]])
            s_3d = ap_of(s, [[1, OC], [OC, OD]])
            sc_3d = ap_of(sc2, [[1, OC], [0, OD]])
            nc.vector.tensor_mul(v_3d, s_3d, sc_3d)
            v_b = smalls.tile([BC, JD], bf, tag="sq_vb")
            nc.vector.tensor_copy(v_b[:], v[:])
            return v, v_b

        def make_vrep(v_b):
            vp = pv_pool.tile([128, JD], f32, tag="vp")
            nc.tensor.matmul(vp[:], bcw[:], v_b[:], start=True, stop=True)
            nc.vector.tensor_copy(vrep[:], vp[:])

        def fold_sp(sacch):
            """((dj-half, b) PSUM accumulator pair) -> [32, 160] PSUM via
            two PE transposes (copy through SBUF f32). The halves live in
            separate PSUM tiles: interleaved accumulation chains in one bank
            corrupt each other."""
            ssb = smalls.tile([80, 2, 32], f32, tag="ssb")
            nc.vector.tensor_copy(ssb[:, 0, :], sacch[0][:])
            nc.vector.tensor_copy(ssb[:, 1, :], sacch[1][:])
            sp = pv_pool.tile([BC, JD], f32, tag="sp")
            for h in (0, 1):
                nc.tensor.transpose(
                    sp[:, 80 * h : 80 * h + 80], ssb[:, h, :], id80[:]
                )
            return sp

        def sacc_pair():
            return (
                ps_pool.tile([80, BC], f32, tag="sacc_a", name="sacc_a"),
                ps_pool.tile([80, BC], f32, tag="sacc_b", name="sacc_b"),
            )

        # ---------- fused phase 1 + iteration 1 ----------
        # s-accumulators live as [80, 2, 32] = ((d,j) half, half, b): the fold
        # matmuls use t2/w9 as lhsT and a 32-wide rhs, so each feeds 32 rows
        # instead of 160 (2.5x less PE time); one transpose pair per squash
        # converts back to [32, 160].
        s0h = sacc_pair()
        sacc1 = None
        st_e = {}
        st_t1 = {}
        st_t2 = {}

        def umm_group(q, g, s0_span):
            """3 u-matmuls into one PSUM group + optional s0 matmuls; evac."""
            pt = pe_pool.tile([128, 3, JD], f32)
            for s in range(3):
                I = q * CB + 3 * g + s
                w, step = divmod(I, WB)
                sl = slice(32 * w, 32 * w + 32)
                nc.tensor.matmul(
                    pt[:, s, :],
                    xblk[sl, step, :],
                    wrhs[sl, step, :],
                    start=True,
                    stop=True,
                    tile_position=(32 * w, 0),
                )
            for kq in s0_span:
                for h in (0, 1):
                    nc.tensor.matmul(
                        s0h[h][:],
                        w9[:, kq, 80 * h : 80 * h + 80],
                        xt9[:, kq, :],
                        start=(kq == 0),
                        stop=(kq == KQ - 1),
                    )
            dst = uhat[:, q * CB + 3 * g : q * CB + 3 * g + 3, :]
            eng = EVAC_PAT[g]
            if eng == "A":
                nc.scalar.copy(dst, pt[:])
            else:
                nc.vector.tensor_copy(dst, pt[:])

        def front_A(q):
            I0 = q * CB
            t1 = scratch.tile([128, CB, JD], bf, tag="sc")
            vr_b = ap_of(vrep, [[0, CB], [1, JD]])
            nc.vector.tensor_mul(t1[:], uhat[:, I0 : I0 + CB, :], vr_b)
            st_t1[q] = t1

        def front_tree(q):
            t1v = st_t1[q][:].rearrange("p c (d j) -> p c d j", d=OD)
            nc.vector.tensor_add(t1v[:, :, 0:8, :], t1v[:, :, 0:8, :], t1v[:, :, 8:16, :])
            nc.vector.tensor_add(t1v[:, :, 0:4, :], t1v[:, :, 0:4, :], t1v[:, :, 4:8, :])

        def front_tail(q, t):
            t1v = st_t1.pop(q)[:].rearrange("p c (d j) -> p c d j", d=OD)
            nc.gpsimd.tensor_add(t1v[:, :, 0:2, :], t1v[:, :, 0:2, :], t1v[:, :, 2:4, :])
            nc.gpsimd.tensor_add(t1v[:, :, 0:1, :], t1v[:, :, 0:1, :], t1v[:, :, 1:2, :])
            rsl = r_bf[:, q * CB : q * CB + CB, :]
            if t == 1:
                nc.gpsimd.tensor_copy(rsl, t1v[:, :, 0, :])
            else:
                nc.gpsimd.tensor_add(rsl, rsl, t1v[:, :, 0, :])
            e_t = sm2.tile([128, CB, OC], bf, tag="e")
            nc.scalar.activation(e_t[:], rsl, AF.Exp)
            st_e[q] = e_t

        def back_zrec(q):
            e_t = st_e[q]
            z_t = sm2.tile([128, CB], f32, tag="z")
            nc.vector.reduce_sum(z_t[:], e_t[:], axis=X)
            nc.vector.reciprocal(z_t[:], z_t[:])
            return z_t

        def back_c(q, z_t):
            e_t = st_e.pop(q)
            c_t = sm2.tile([128, CB, OC], bf, tag="c")
            z_b = ap_of(z_t, [[1, CB], [0, OC]])
            nc.gpsimd.tensor_mul(c_t[:], e_t[:], z_b)
            return c_t

        def back_Bhead(q, c_t, pi):
            t2 = scratch.tile([128, CB, JD], bf, tag="sc")
            u4 = uhat[:, q * CB : q * CB + CB, :].rearrange("p c (d j) -> p c d j", d=OD)
            t24 = t2[:].rearrange("p c (d j) -> p c d j", d=OD)
            c_p = ap_of(c_t, [[OC, pi], [0, OD], [1, OC]])
            nc.gpsimd.tensor_mul(t24[:, 0:pi], u4[:, 0:pi], c_p)
            st_t2[q] = (t2, c_t)

        def back_Btail(q, pi):
            t2, c_t = st_t2[q]
            u4 = uhat[:, q * CB : q * CB + CB, :].rearrange("p c (d j) -> p c d j", d=OD)
            t24 = t2[:].rearrange("p c (d j) -> p c d j", d=OD)
            c_d = ap_of(c_t, [[OC, CB - pi], [0, OD], [1, OC]], extra_off=pi * OC)
            nc.vector.tensor_mul(t24[:, pi:CB], u4[:, pi:CB], c_d)

        def s_folds(q, sacc):
            t2, _ = st_t2.pop(q)
            for s in range(CB):
                gi = q * CB + s
                for h in (0, 1):
                    nc.tensor.matmul(
                        sacc[h][:],
                        t2[:, s, 80 * h : 80 * h + 80],
                        onesb[:],
                        start=(gi == 0),
                        stop=(gi == NB - 1),
                    )

        # Prelude BEFORE any u-matmul is emitted: the Tile scheduler pops the
        # lowest-priority READY instruction at every engine-idle, so the
        # s0 -> squash -> vrep chain must outrank the evac-gated u-matmul
        # backlog or the fronts stall behind ~50us of PE flood.
        for kq in range(KQ):
            for h in (0, 1):
                nc.tensor.matmul(
                    s0h[h][:],
                    w9[:, kq, 80 * h : 80 * h + 80],
                    xt9[:, kq, :],
                    start=(kq == 0),
                    stop=(kq == KQ - 1),
                )
        v_f, v_b = squash(fold_sp(s0h), 0.1)
        make_vrep(v_b)
        sacc1 = sacc_pair()

        # body b: PE u-mms(b)+evac(b) [ACT]; DVE [A(b-2), Z(b-3), rec(b-3),
        # B-tail(b-3), h8(b-2), h4(b-2)]; Pool [c(b-3), B-head(b-3), h2(b-2),
        # h1(b-2), r(b-2)]; ACT exp(b-2); PE s-folds(b-3). Every cross-engine
        # input is >= 1 body old except rec->c and c->B-tail (short waits).
        for body in range(NCHUNK + 3):
            if body < NCHUNK:
                for g in range(NG):
                    umm_group(body, g, ())
            if 2 <= body <= NCHUNK + 1:
                front_A(body - 2)
            if 3 <= body <= NCHUNK + 2:
                qb = body - 3
                z_t = back_zrec(qb)
                c_t = back_c(qb, z_t)
                back_Bhead(qb, c_t, PI1)
                back_Btail(qb, PI1)
                s_folds(qb, sacc1)
            if 2 <= body <= NCHUNK + 1:
                front_tree(body - 2)
                front_tail(body - 2, t=1)

        v_f, v_b = squash(fold_sp(sacc1), 1.0)
        make_vrep(v_b)

        # ---------- iteration 2 ----------
        sacc2 = sacc_pair()
        for body in range(NCHUNK + 2):
            if body < NCHUNK:
                front_A(body)
            if body >= 2:
                qb = body - 2
                z_t = back_zrec(qb)
                c_t = back_c(qb, z_t)
                back_Bhead(qb, c_t, PI2)
                back_Btail(qb, PI2)
                s_folds(qb, sacc2)
            if body < NCHUNK:
                front_tree(body)
                front_tail(body, t=2)

        v_f, v_b = squash(fold_sp(sacc2), 1.0)

        # ---------- output ----------
        stage = smalls.tile([BC, JD], f32, tag="stage")
        st_v = ap_of(stage, [[OD, OC], [1, OD]])  # (j outer, d inner) dense
        vf_v = ap_of(v_f, [[1, OC], [OC, OD]])
        nc.vector.tensor_copy(st_v, vf_v)
        nc.sync.dma_start(d_y[:], stage[:].rearrange("p (j d) -> p j d", j=OC))

    nc.compile()
    return nc


def _prep_host(x, W):
    """Build per-core input maps. x [256,1152,8] f32, W [1152,10,8,16] f32."""
    Wv = W.reshape(NW, WB, 4, OC, ID, OD)  # [w, step, ip, j, k, d]
    wrhs = np.ascontiguousarray(
        Wv.transpose(0, 2, 4, 1, 5, 3).reshape(128, WB, JD)
    ).astype(BF16)
    Wf = W.transpose(0, 2, 3, 1).reshape(IC * ID, OD, OC)  # [(i,k), d, j]
    w9 = np.ascontiguousarray(
        Wf.reshape(KQ, 128, OD, OC).transpose(1, 0, 2, 3).reshape(128, KQ, JD)
    ).astype(BF16)
    onesb = np.zeros((128, BC), dtype=BF16)
    onesb[np.arange(128), np.arange(128) % 32] = 1
    bcw = np.zeros((BC, 128), dtype=BF16)
    bcw[np.arange(128) % 32, np.arange(128)] = 1
    id80 = np.eye(80, dtype=np.float32)

    in_maps = []
    for c in range(NCORES):
        xc = x[c * BC : (c + 1) * BC]  # [32, 1152, 8]
        xv = xc.reshape(BC, NW, WB, 4, ID)  # [b, w, step, ip, k]
        xa = np.zeros((NW, 4, ID, WB, 4, BC), dtype=BF16)  # [w, ip, k, step, ipp, b]
        for ip in range(4):
            xa[:, ip, :, :, ip, :] = xv[:, :, :, ip, :].transpose(1, 3, 2, 0)
        xblk = xa.reshape(128, WB, 128)
        xf = xc.transpose(1, 2, 0).reshape(IC * ID, BC)  # [(i,k), b]
        xt9 = np.ascontiguousarray(
            xf.reshape(KQ, 128, BC).transpose(1, 0, 2)
        ).astype(BF16)
        in_maps.append(
            {
                "xblk": xblk,
                "wrhs": wrhs,
                "w9": w9,
                "xt9": xt9,
                "onesb": onesb,
                "bcw": bcw,
                "id80": id80,
            }
        )
    return in_maps


def kernel(x, W, _trace=False, _trace_kwargs=None):
    from concourse.bass_utils import run_bass_kernel_spmd

    x = np.asarray(x, dtype=np.float32)
    W = np.asarray(W, dtype=np.float32)
    if "nc" not in _CACHE:
        _CACHE["nc"] = _build_nc()
    nc = _CACHE["nc"]
    in_maps = _prep_host(x, W)
    res = run_bass_kernel_spmd(
        nc,
        in_maps,
        core_ids=list(range(NCORES)),
        trace=_trace,
        **(_trace_kwargs or {}),
    )
    _CACHE["last_results"] = res
    out = np.concatenate([res.results[c]["y"] for c in range(NCORES)], axis=0)
    return out
